# revision 10
# baseline (speedup 1.0000x reference)
"""AdaGAE forward on 8 TRN2 NeuronCores.

Computes, for A = norm_adj_matrix [8192, 8192]:
    h    = relu(A @ (X @ W1))           [n, 24]
    emb  = A @ (h @ W2)                 [n, 12]
    out  = softmax(-pairwise_sq_dists(emb), axis=1) + 1e-10

Key algebra: softmax is shift-invariant per row and the relu on squared
distances is a numerical no-op, so
    out[i, :] = softmax_j(2*<emb_i, emb_j> - |emb_j|^2) + eps
with the row-constant |emb_i|^2 cancelled. The -|emb_j|^2 bias is folded
into the Gram matmul as two extra contraction rows (ones x -sq_hi/lo), so
the whole logits matrix comes out of the PE in one k=38 fp16 hi/lo-split
matmul per tile.

Distribution: row-shard A across 8 cores. Each core stages its shard as
A_shard.T in fp16 (fits SBUF entirely -> A is read from HBM exactly once),
computes hT/embT for its rows, and two tiny AllGathers (hW2, then emb+sq)
make the full embedding visible everywhere. fp16 is used for the big
streams (bf16 is too coarse for the softmax logits); all stationary
operands are fp16 hi+lo split (error ~2^-21) and accumulate in f32 PSUM.
"""
import sys

for _p in ("/opt/trn_rl_repo", "/opt/trn_rl_repo/concourse"):
    if _p not in sys.path:
        sys.path.insert(0, _p)

import numpy as np

from concourse import bacc, mybir, tile
from concourse import bass_utils
from concourse.masks import make_identity

F32 = mybir.dt.float32
F16 = mybir.dt.float16
BF16 = mybir.dt.bfloat16
ADD = mybir.AluOpType.add
SUB = mybir.AluOpType.subtract
MULT = mybir.AluOpType.mult
AF = mybir.ActivationFunctionType

N = 8192
NCORES = 8
SH = N // NCORES          # 1024 rows per core
P = 128
KB = N // P               # 64 contraction blocks
D_IN, D_MID, D_EMB = 32, 24, 12
KAUG = 98                 # 32-aligned rows: 0:12=2e_hi, 32:44=2e_hi, 64:76=2e_lo, 96:98=ones
EPS = 1e-10
RG = [list(range(NCORES))]

_NC = None


def _emit(nc, tc, a_t, xt, w1, w2, out):
    dram_cm = tc.tile_pool(name="dram", bufs=1, space="DRAM")
    dram = dram_cm.__enter__()
    sbp_cm = tc.tile_pool(name="sbp", bufs=1)
    sbp = sbp_cm.__enter__()

    # persists into phase 3: the Gram matmul's stationary operand
    lhsT_sb = sbp.tile([KAUG, SH], F16, name="lhsT_sb")
    nc.vector.memset(lhsT_sb[:], 0.0)


    with tc.tile_pool(name="s12", bufs=1) as s12:
        # ---- small inputs ----
        w1_sb = s12.tile([2 * D_IN, 2 * D_MID], F16, name="w1_sb")
        nc.sync.dma_start(w1_sb[:], w1[:])
        w2_sb = s12.tile([D_MID, D_EMB], F32, name="w2_sb")
        nc.sync.dma_start(w2_sb[:], w2[:])
        ident = s12.tile([P, P], F32, name="ident")
        make_identity(nc, ident)
        neg_ones = s12.tile([D_EMB, 1], F32, name="neg_ones")
        nc.vector.memset(neg_ones[:], -1.0)

        # ---- the A shard, resident in SBUF for both A-matmuls ----
        a_tiles = []
        for kb in range(KB):
            at = s12.tile([P, SH], F16, name=f"a{kb}", tag=f"a{kb}")
            nc.sync.dma_start(at[:], a_t[kb * P:(kb + 1) * P, :])
            a_tiles.append(at)

        # ---- XW1 = X @ W1, f32 via fp16 hi/lo on both sides ----
        xw1aug = s12.tile([P, KB, 64], F16, name="xw1aug")
        nc.vector.memset(xw1aug[:], 0.0)
        hT = s12.tile([D_MID, SH], F32, name="hT")
        with tc.tile_pool(name="p1a", bufs=1, space="PSUM") as p1a:
            scr32 = s12.tile([P, KB * D_MID], F32, name="scr32", tag="scr32")
            scr32_v = scr32[:].rearrange("p (k m) -> p k m", m=D_MID)
            for g in range(8):
                xps = p1a.tile([P, 8 * D_MID], F32, name="xw1ps",
                               tag="xw1ps", bufs=2)
                xps_v = xps[:].rearrange("p (j m) -> p j m", m=D_MID)
                for j in range(8):
                    kb = g * 8 + j
                    xtc = s12.tile([2 * D_IN, P], F16, name=f"xtc{kb}",
                                   tag="xtc", bufs=4)
                    nc.sync.dma_start(xtc[:], xt[:, kb * P:(kb + 1) * P])
                    # X@W1_hi + X@W1_lo accumulated in PSUM
                    nc.tensor.matmul(
                        xps[:, j * D_MID:(j + 1) * D_MID],
                        lhsT=xtc[:], rhs=w1_sb[:, 0:D_MID],
                        start=True, stop=False, skip_group_check=True)
                    nc.tensor.matmul(
                        xps[:, j * D_MID:(j + 1) * D_MID],
                        lhsT=xtc[:], rhs=w1_sb[:, D_MID:2 * D_MID],
                        start=False, stop=True, skip_group_check=True)
                gsl = slice(g * 8, (g + 1) * 8)
                # fp16 hi/lo split of XW1 -> mm1's stationary operand
                nc.scalar.copy(xw1aug[:, gsl, 0:D_MID], xps_v)
                nc.scalar.copy(scr32_v[:, gsl, :], xw1aug[:, gsl, 0:D_MID])
                nc.vector.tensor_tensor(xw1aug[:, gsl, 32:32 + D_MID],
                                        xps_v, scr32_v[:, gsl, :], SUB)

            # ---- mm1: hT_aug = (XW1 hi/lo).T @ A_shard.T, chasing A DMAs ----
            hps = [p1a.tile([64, 512], F32, name=f"hps{h}",
                            tag=f"hps{h}") for h in range(2)]
            for kb in range(KB):
                for h in range(2):
                    nc.tensor.matmul(
                        hps[h][:], lhsT=xw1aug[:, kb, :],
                        rhs=a_tiles[kb][:, h * 512:(h + 1) * 512],
                        start=(kb == 0), stop=(kb == KB - 1),
                        skip_group_check=True)

            # hT = relu(hi + lo)   [24, 1024] f32
            hlo = s12.tile([D_MID, SH], F32, name="hlo")
            for h in range(2):
                nc.scalar.copy(hlo[:, h * 512:(h + 1) * 512],
                               hps[h][32:32 + D_MID, :])
                nc.vector.tensor_tensor(hT[:, h * 512:(h + 1) * 512],
                                        hps[h][0:D_MID, :],
                                        hlo[:, h * 512:(h + 1) * 512], ADD)
            nc.vector.tensor_scalar_max(hT[:], hT[:], 0.0)

        # ---- hW2 (exact f32), then transpose to natural layout for AG1 ----
        ag1_sb = s12.tile([P, 8 * D_EMB], F32, name="ag1_sb")
        hw2T = s12.tile([D_EMB, SH], F32, name="hw2T")
        with tc.tile_pool(name="p1b", bufs=1, space="PSUM") as p1b:
            h2ps = p1b.tile([D_EMB, SH], F32, name="h2ps")
            for c in range(2):
                nc.tensor.matmul(h2ps[:, c * 512:(c + 1) * 512],
                                 lhsT=w2_sb[:],
                                 rhs=hT[:, c * 512:(c + 1) * 512],
                                 start=True, stop=True)
            nc.scalar.copy(hw2T[:], h2ps[:])
            h2nat_ps = p1b.tile([P, 8 * D_EMB], F32, name="h2nat_ps")
            for q in range(8):
                nc.tensor.transpose(h2nat_ps[:, q * D_EMB:(q + 1) * D_EMB],
                                    hw2T[:, q * P:(q + 1) * P],
                                    ident[0:D_EMB, 0:D_EMB])
            nc.scalar.copy(ag1_sb[:], h2nat_ps[:])

        ag1_in = dram.tile([SH, D_EMB], F32, name="ag1_in")
        nc.sync.dma_start(
            ag1_in[:].rearrange("(q p) d -> p q d", p=P),
            ag1_sb[:].rearrange("p (q d) -> p q d", d=D_EMB))
        ag1_out = dram.tile([NCORES, SH, D_EMB], F32, name="ag1_out")
        nc.gpsimd.collective_compute(
            "AllGather", mybir.AluOpType.bypass, replica_groups=RG,
            ins=[ag1_in[:].opt()], outs=[ag1_out[:].opt()])

        # ---- full hW2 -> fp16 hi/lo stationary operand for mm2 ----
        hw2n = s12.tile([P, KB * D_EMB], F32, name="hw2n")
        nc.sync.dma_start(
            hw2n[:].rearrange("p (r q d) -> p r q d", r=NCORES, q=8),
            ag1_out[:].rearrange("r (q p) d -> p r q d", p=P))
        hw2n_v = hw2n[:].rearrange("p (k d) -> p k d", d=D_EMB)
        hw2aug = s12.tile([P, KB, 48], F16, name="hw2aug")
        nc.vector.memset(hw2aug[:], 0.0)
        nc.scalar.copy(hw2aug[:, :, 0:D_EMB], hw2n_v)
        scr32b = s12.tile([P, KB * D_EMB], F32, name="scr32b", tag="scr32")
        scr32b_v = scr32b[:].rearrange("p (k d) -> p k d", d=D_EMB)
        nc.scalar.copy(scr32b_v, hw2aug[:, :, 0:D_EMB])
        nc.vector.tensor_tensor(hw2aug[:, :, 32:32 + D_EMB],
                                hw2n_v, scr32b_v, SUB)

        # ---- mm2: embT_aug = (hW2 hi/lo).T @ A_shard.T (A from SBUF) ----
        embT = s12.tile([D_EMB, SH], F32, name="embT")
        e_hi = s12.tile([D_EMB, SH], F16, name="e_hi")
        e_lo = s12.tile([D_EMB, SH], F16, name="e_lo")
        msq_hi = s12.tile([1, SH], F16, name="msq_hi")
        msq_lo = s12.tile([1, SH], F16, name="msq_lo")
        with tc.tile_pool(name="p2", bufs=1, space="PSUM") as p2:
            eps_ = [p2.tile([48, 512], F32, name=f"eps{h}",
                            tag=f"eps{h}") for h in range(2)]
            for kb in range(KB):
                for h in range(2):
                    nc.tensor.matmul(
                        eps_[h][:], lhsT=hw2aug[:, kb, :],
                        rhs=a_tiles[kb][:, h * 512:(h + 1) * 512],
                        start=(kb == 0), stop=(kb == KB - 1),
                        skip_group_check=True)
            elo_s = s12.tile([D_EMB, SH], F32, name="elo_s")
            for h in range(2):
                nc.scalar.copy(elo_s[:, h * 512:(h + 1) * 512],
                               eps_[h][32:32 + D_EMB, :])
                nc.vector.tensor_tensor(embT[:, h * 512:(h + 1) * 512],
                                        eps_[h][0:D_EMB, :],
                                        elo_s[:, h * 512:(h + 1) * 512], ADD)

            # ---- emb fp16 hi/lo + local Gram lhsT rows ----
            nc.scalar.copy(e_hi[:], embT[:])
            e_hi32 = s12.tile([D_EMB, SH], F32, name="e_hi32")
            nc.scalar.copy(e_hi32[:], e_hi[:])
            nc.vector.tensor_tensor(e_lo[:], embT[:], e_hi32[:], SUB)
            nc.scalar.mul(lhsT_sb[0:D_EMB, :], e_hi[:], 2.0)
            nc.scalar.mul(lhsT_sb[32:32 + D_EMB, :], e_hi[:], 2.0)
            nc.scalar.mul(lhsT_sb[64:64 + D_EMB, :], e_lo[:], 2.0)
            nc.vector.memset(lhsT_sb[96:98, :], 1.0)

            # ---- -|emb_j|^2, fp16 hi/lo ----
            sqel = s12.tile([D_EMB, SH], F32, name="sqel")
            nc.vector.tensor_mul(sqel[:], embT[:], embT[:])
            msq_ps = p2.tile([1, SH], F32, name="msq_ps")
            for c in range(2):
                nc.tensor.matmul(msq_ps[:, c * 512:(c + 1) * 512],
                                 lhsT=neg_ones[:],
                                 rhs=sqel[:, c * 512:(c + 1) * 512],
                                 start=True, stop=True)
            nc.scalar.copy(msq_hi[:], msq_ps[:])
            msq_hi32 = s12.tile([1, SH], F32, name="msq_hi32")
            nc.scalar.copy(msq_hi32[:], msq_hi[:])
            nc.vector.tensor_tensor(msq_lo[:], msq_ps[:], msq_hi32[:], SUB)

        # ---- AllGather #2: emb hi/lo + (-sq) hi/lo, all fp16 ----
        ag2_in = dram.tile([2 * D_EMB + 2, SH], F16, name="ag2_in")
        nc.sync.dma_start(ag2_in[0:D_EMB, :], e_hi[:])
        nc.sync.dma_start(ag2_in[D_EMB:2 * D_EMB, :], e_lo[:])
        nc.sync.dma_start(ag2_in[2 * D_EMB:2 * D_EMB + 1, :], msq_hi[:])
        nc.sync.dma_start(ag2_in[2 * D_EMB + 1:2 * D_EMB + 2, :], msq_lo[:])
        ag2_out = dram.tile([NCORES, 2 * D_EMB + 2, SH], F16, name="ag2_out")
        nc.gpsimd.collective_compute(
            "AllGather", mybir.AluOpType.bypass, replica_groups=RG,
            ins=[ag2_in[:].opt()], outs=[ag2_out[:].opt()])

    # ---- phase 3: logits -> exp -> row-normalize -> out ----
    with tc.tile_pool(name="s3", bufs=1) as s3, \
         tc.tile_pool(name="p3", bufs=1, space="PSUM") as p3:
        rhs_sb = s3.tile([KAUG, N], F16, name="rhs_sb")
        nc.vector.memset(rhs_sb[:], 0.0)
        nc.sync.dma_start(
            rhs_sb[0:D_EMB, :].rearrange("d (r j) -> d r j", r=NCORES),
            ag2_out[:, 0:D_EMB, :].rearrange("r d j -> d r j"))
        nc.sync.dma_start(
            rhs_sb[32:32 + D_EMB, :].rearrange("d (r j) -> d r j", r=NCORES),
            ag2_out[:, D_EMB:2 * D_EMB, :].rearrange("r d j -> d r j"))
        nc.sync.dma_start(
            rhs_sb[64:64 + D_EMB, :].rearrange("d (r j) -> d r j", r=NCORES),
            ag2_out[:, 0:D_EMB, :].rearrange("r d j -> d r j"))
        nc.sync.dma_start(
            rhs_sb[96:98, :].rearrange("d (r j) -> d r j", r=NCORES),
            ag2_out[:, 2 * D_EMB:2 * D_EMB + 2, :].rearrange("r d j -> d r j"))

        for mt in range(SH // P):
            t_sb = s3.tile([P, N], BF16, name="t_sb", tag="t_sb", bufs=2)
            acc = s3.tile([P, 4], F32, name="acc", tag="acc", bufs=2)
            for g in range(4):
                zps = p3.tile([P, 2048], F32, name="zps", tag="zps", bufs=2)
                for c in range(4):
                    nc.tensor.matmul(
                        zps[:, c * 512:(c + 1) * 512],
                        lhsT=lhsT_sb[:, mt * P:(mt + 1) * P],
                        rhs=rhs_sb[:, g * 2048 + c * 512:g * 2048 + (c + 1) * 512],
                        start=True, stop=True)
                nc.scalar.activation(
                    t_sb[:, g * 2048:(g + 1) * 2048], zps[:], AF.Exp,
                    accum_out=acc[:, g:g + 1])
            ssum = s3.tile([P, 1], F32, name="ssum", tag="ssum", bufs=2)
            nc.vector.reduce_sum(ssum[:], acc[:], axis=mybir.AxisListType.X)
            recip = s3.tile([P, 1], F32, name="recip", tag="recip", bufs=2)
            nc.vector.reciprocal(recip[:], ssum[:])
            nc.vector.tensor_scalar(t_sb[:], t_sb[:], recip[:], EPS, MULT, ADD)
            nc.sync.dma_start(out[mt * P:(mt + 1) * P, :], t_sb[:])

    sbp_cm.__exit__(None, None, None)
    dram_cm.__exit__(None, None, None)


def _build():
    nc = bacc.Bacc("TRN2", target_bir_lowering=False, debug=False,
                   num_devices=NCORES)
    a_t = nc.dram_tensor("a_t", [N, SH], F16, kind="ExternalInput")
    xt = nc.dram_tensor("xt", [2 * D_IN, N], F16, kind="ExternalInput")
    w1 = nc.dram_tensor("w1", [2 * D_IN, 2 * D_MID], F16, kind="ExternalInput")
    w2 = nc.dram_tensor("w2", [D_MID, D_EMB], F32, kind="ExternalInput")
    out = nc.dram_tensor("out", [SH, N], BF16, kind="ExternalOutput")
    with tile.TileContext(nc) as tc:
        _emit(nc, tc, a_t.ap(), xt.ap(), w1.ap(), w2.ap(), out.ap())
    nc.compile()
    return nc


def _get_nc():
    global _NC
    if _NC is None:
        _NC = _build()
    return _NC


def _prep_in_maps(norm_adj_matrix, X, W1, W2):
    A = np.asarray(norm_adj_matrix, dtype=np.float32)
    X = np.asarray(X, dtype=np.float32)
    W1 = np.asarray(W1, dtype=np.float32)
    W2 = np.asarray(W2, dtype=np.float32)

    Xh = X.astype(np.float16)
    Xl = (X - Xh.astype(np.float32)).astype(np.float16)
    xt = np.ascontiguousarray(np.concatenate([Xh.T, Xl.T], axis=0))
    W1h = W1.astype(np.float16)
    W1l = (W1 - W1h.astype(np.float32)).astype(np.float16)
    w1row = np.concatenate([W1h, W1l], axis=1)
    w1 = np.ascontiguousarray(np.concatenate([w1row, w1row], axis=0))

    in_maps = []
    for c in range(NCORES):
        a_t = A[c * SH:(c + 1) * SH, :].T.astype(np.float16)
        in_maps.append({"a_t": a_t, "xt": xt, "w1": w1, "w2": W2})
    return in_maps


def _execute(in_maps, trace=False, tmpdir=None):
    return bass_utils.run_bass_kernel_spmd(
        _get_nc(), in_maps, core_ids=list(range(NCORES)),
        trace=trace, tmpdir=tmpdir)


def _assemble(res):
    shards = [np.asarray(res.results[c]["out"]).astype(np.float32)
              for c in range(NCORES)]
    return np.concatenate(shards, axis=0)


def kernel(norm_adj_matrix, X, W1, W2):
    in_maps = _prep_in_maps(norm_adj_matrix, X, W1, W2)
    res = _execute(in_maps)
    return _assemble(res)
